# revision 5
# baseline (speedup 1.0000x reference)
"""Box-from-mask kernel for Trainium2 (8 NeuronCores, SPMD data-parallel).

Problem: masks [100, 800, 1280] f32 -> boxes [100, 2, 2] f32 where
box[n] = [[xmin, ymin], [xmax, ymax]] of {(y, x) : masks[n, y, x] > 0.5},
with empty-mask sentinels xmin=W, ymin=H, xmax=-1, ymax=-1.

Flat-row sharding: the 100*800 = 80,000 mask rows are treated as one flat
[80000, 1280] array. Core c owns rows [c*9984, c*9984+10112): 79 uniform
128-row tiles per core, no runt DMAs anywhere. Shards overlap by 128 rows
(cores 0-6 re-read one tile of their neighbour, +1.1% traffic); duplicated
rows are harmless because every quantity is max/union-combined host-side.

Per-core device pipeline, per [128, 1280] block:
  - one DVE tensor_scalar(is_gt 0.5) -> 0/1 bf16 block, with accum_out(max)
    giving the per-row "any pixel" bit (one elementwise pass per element).
  - PE selector matmul accumulates per-column counts into a [14, 1280] PSUM
    region; the [128, 14] one-hot selector for each block routes every SBUF
    partition (= one mask row) to its mask's PSUM row, so blocks that
    straddle a mask boundary need no special casing.
Blocks arrive in partition-major DMA groups (8 tiles -> one 5.2 MB DMA,
each partition reading 40 KB contiguous) alternating between the SP and
ACT HWDGE rings; the final groups are small (4+3 tiles) so the trailing
compute after the last transfer is short. Outputs are the raw row-any
bits [128, 79] and column presence [14, 1280]; the min/max index
arithmetic happens host-side on ~30 KB per core.
"""

import sys

for _p in ("/opt/trn_rl_repo", "/opt/pypackages"):
    if _p not in sys.path:
        sys.path.append(_p)

import ml_dtypes
import numpy as np

import concourse.bass as bass
import concourse.tile as tile
from concourse import bacc, mybir
from concourse.bass_utils import run_bass_kernel_spmd

N, H, W = 100, 800, 1280
N_CORES = 8
THRESHOLD = 0.5

ROWS = N * H  # 80,000 flat rows
STRIDE = 9984  # 78 tiles; core c starts at c*STRIDE
SHARD_ROWS = 10112  # 79 tiles actually read (one tile overlap)
NB = SHARD_ROWS // 128  # 79 blocks of 128 rows
GSZ = 14  # max distinct masks touched by one core's shard
# DMA groups: sizes of consecutive tile groups fetched by one DMA each.
# Tapered tail: the short final groups shrink the serial compute left
# after the last transfer lands.
GROUP_SIZES = [8] * 9 + [4, 2, 1]
assert sum(GROUP_SIZES) == NB

fp32 = mybir.dt.float32
fp16 = mybir.dt.float16
bf16 = mybir.dt.bfloat16
Op = mybir.AluOpType


def _chunks(w):
    return [(c, min(512, w - c)) for c in range(0, w, 512)]


def _groups():
    """[(row_offset, n_tiles)] per DMA group."""
    out, r = [], 0
    for t in GROUP_SIZES:
        out.append((r, t))
        r += t * 128
    return out


def _local_rows():
    """local_rows[p, B] = shard-local row held by partition p for block B.

    Partition-major DMA: group (R, T) lands rows R + p*T + a on partition p,
    column-block a.
    """
    cols = []
    for R, T in _groups():
        for a in range(T):
            cols.append(R + np.arange(128) * T + a)
    return np.stack(cols, axis=1)  # [128, NB]


LOCAL_ROWS = _local_rows()

RAW_BUFS = 3
BIN_BUFS = 8


def build_program():
    """One-core Bass/Tile program; run SPMD on all 8 cores."""
    chunks = _chunks(W)
    groups = _groups()
    tmax = max(t for _, t in groups)

    nc = bacc.Bacc(
        "TRN2", target_bir_lowering=False, debug=False, enable_asserts=False
    )
    masks = nc.dram_tensor("masks", [SHARD_ROWS, W], fp32, kind="ExternalInput").ap()
    sel = nc.dram_tensor("sel", [128, NB * GSZ], bf16, kind="ExternalInput").ap()
    rowany_out = nc.dram_tensor(
        "rowany_out", [128, NB], fp32, kind="ExternalOutput"
    ).ap()
    counts_out = nc.dram_tensor(
        "counts_out", [GSZ, W], fp16, kind="ExternalOutput"
    ).ap()

    with tile.TileContext(nc) as tc:
        with (
            tc.tile_pool(name="raw", bufs=RAW_BUFS) as rawp,
            tc.tile_pool(name="bin", bufs=BIN_BUFS) as binp,
            tc.tile_pool(name="consts", bufs=1) as constp,
            tc.tile_pool(name="psum", bufs=1, space="PSUM") as psump,
        ):
            # selector rides the gpsimd SWDGE queue so the SP/ACT HWDGE
            # queues start streaming mask tiles immediately
            sel_t = constp.tile([128, NB * GSZ], bf16)
            nc.gpsimd.dma_start(sel_t[:], sel)
            rowany = constp.tile([128, NB], fp32)
            nc.gpsimd.memset(rowany[:], 0.0)
            csb = constp.tile([GSZ, W], fp16)
            cc = psump.tile([GSZ, W], fp32, name="cc", tag="cc")

            b_idx = 0
            for gi, (R, T) in enumerate(groups):
                raw = rawp.tile([128, tmax * W], fp32, tag="raw")
                eng = nc.sync if gi % 2 == 0 else nc.scalar
                eng.dma_start(
                    raw[:, : T * W],
                    masks[R : R + 128 * T, :].rearrange("(p a) x -> p (a x)", p=128),
                )
                for a in range(T):
                    b = binp.tile([128, W], bf16, tag="b")
                    nc.vector.tensor_scalar(
                        out=b[:],
                        in0=raw[:, a * W : (a + 1) * W],
                        scalar1=THRESHOLD,
                        scalar2=None,
                        op0=Op.is_gt,
                        op1=Op.max,
                        accum_out=rowany[:, b_idx : b_idx + 1],
                    )
                    for c0, cw in chunks:
                        nc.tensor.matmul(
                            cc[:, c0 : c0 + cw],
                            sel_t[:, b_idx * GSZ : b_idx * GSZ + GSZ],
                            b[:, c0 : c0 + cw],
                            start=(b_idx == 0),
                            stop=(b_idx == NB - 1),
                        )
                    b_idx += 1

            # (count > 0) -> fp16 presence bits, then tiny output DMAs.
            nc.vector.tensor_scalar(
                out=csb[:],
                in0=cc[:, :],
                scalar1=0.0,
                scalar2=None,
                op0=Op.is_gt,
            )
            nc.sync.dma_start(counts_out, csb[:])
            nc.gpsimd.dma_start(rowany_out, rowany[:])

    nc.compile()
    return nc


def make_sel(core):
    """Per-block one-hot selector: partition p -> local mask index."""
    g = core * STRIDE + LOCAL_ROWS  # [128, NB] global rows
    first = (core * STRIDE) // H
    ul = g // H - first
    assert ul.min() >= 0 and ul.max() < GSZ
    sel = np.zeros((128, NB * GSZ), ml_dtypes.bfloat16)
    sel[np.arange(128)[:, None], np.arange(NB)[None, :] * GSZ + ul] = 1
    return sel


def postprocess(results):
    """Per-core rowany/counts -> boxes [N, 2, 2] f32 (exact)."""
    v1 = np.zeros(N)  # H - ymin   (0 if empty)
    v2 = np.zeros(N)  # ymax + 1
    u1 = np.zeros(N)  # W - xmin
    u2 = np.zeros(N)  # xmax + 1
    xs = np.arange(W)
    for c, r in enumerate(results):
        g = c * STRIDE + LOCAL_ROWS
        unit = g // H
        y = g % H
        a = np.asarray(r["rowany_out"]) > 0
        np.maximum.at(v1, unit[a], (H - y)[a])
        np.maximum.at(v2, unit[a], (y + 1)[a])
        first = (c * STRIDE) // H
        nu = (c * STRIDE + SHARD_ROWS - 1) // H - first + 1
        p = np.asarray(r["counts_out"][:nu]) > 0  # [nu, W]
        np.maximum.at(u1, first + np.arange(nu), np.max(np.where(p, W - xs, 0), 1))
        np.maximum.at(u2, first + np.arange(nu), np.max(np.where(p, xs + 1, 0), 1))
    boxes = np.empty((N, 2, 2), np.float32)
    boxes[:, 0, 0] = W - u1  # xmin
    boxes[:, 0, 1] = H - v1  # ymin
    boxes[:, 1, 0] = u2 - 1  # xmax
    boxes[:, 1, 1] = v2 - 1  # ymax
    return boxes


_cache = {}


def _get_program():
    if "nc" not in _cache:
        _cache["nc"] = build_program()
        _cache["sel"] = [make_sel(c) for c in range(N_CORES)]
    return _cache["nc"], _cache["sel"]


def make_in_maps(masks):
    masks = np.ascontiguousarray(np.asarray(masks, dtype=np.float32))
    _, sels = _get_program()
    flat = masks.reshape(ROWS, W)
    return [
        {"masks": flat[c * STRIDE : c * STRIDE + SHARD_ROWS], "sel": sels[c]}
        for c in range(N_CORES)
    ]


def kernel(masks):
    nc, _ = _get_program()
    in_maps = make_in_maps(masks)
    res = run_bass_kernel_spmd(nc, in_maps, core_ids=list(range(N_CORES)))
    return postprocess(res.results)
